# revision 21
# baseline (speedup 1.0000x reference)
"""LRU forward on 8 Trainium2 NeuronCores.

Sharding: 8 shards = 4 batches x 2 sequence halves (L_local = 2048).
Per-core dataflow is fully transposed (channels on SBUF partitions, time on
the free dim), bf16 matmul operands, and a CHUNK-LOCAL rotating frame:

  in proj   p = Bg^T @ x            (bf16 matmuls, fp32 PSUM)
  rot-in    v[s] = e^{-i theta s} p[s]   per 512-chunk local time s
            (bf16 DVE tensor_tensor, 2x perf mode)
  scan      w = scan(r, v)          4 real per-lane hw scans, init 0 per chunk
  carry     column-only recurrence c_{m+1} = e^{i theta MC}(W_m + r^MC c_m),
            cross-core end-state exchanged via a 16-col AllReduce
  fix       chunk 0: h-space  h += e^{i theta s} r^{s+1} c
            chunks 1-3: w-space  w += r^{s+1} c
  rot-out   h[s] = e^{+i theta s} w[s]  (bf16)
  out proj  y^T = CT^T @ h + diag(D) blocks @ x^T   (bf16 matmuls)

The chunk-local frame needs only [256, 512] cos/sin tables (vs [256, 2048]
twice), loaded once. All big DMAs are batched bf16 transfers, with the
first x chunk and Bg split in halves so the tensor engine starts early.
"""

import os

import numpy as np

B, L, D, N = 4, 4096, 1024, 256
NCORE = 8
LLOC = L // 2          # per-core sequence length
MC = 512               # time chunk
NMC = LLOC // MC       # 4 chunks
N2 = 2 * N             # stacked re|im channels

_CACHE = {}
LAST_RESULTS = None    # test.py reads exec_time_ns from here

# cf (f32 const pack) column offsets
_CF_RB = 0                     # 2*MC : r broadcast, per c-block
_CF_RMC = 2 * MC               # +c   : r^{MC}
_CF_CMC = 2 * MC + 2           # +c   : cos(theta MC)
_CF_SMC = 2 * MC + 4           # +c   : sin(theta MC)
_CF_SMCN = 2 * MC + 6          # +c   : -sin(theta MC)
_CF_RFC = 2 * MC + 8           # +4c+m: r^{mMC} cos(theta (m+1) MC)  (0 if h=0)
_CF_RFS = 2 * MC + 16          # +4c+m: r^{mMC} sin(theta (m+1) MC)
_CF_RFSN = 2 * MC + 24         # +4c+m: negated RFS
_CF_GM = 2 * MC + 32           # +p   : contribute mask
_CF_PM = 2 * MC + 36           # +p   : select mask
_CF_COLS = 2 * MC + 40

# cb2 (bf16 const pack) column offsets
_C2_RPOW = 0                   # 2*MC : r^{s+1}
_C2_RC = 2 * MC                # 2*MC : r^{s+1} cos(theta s)
_C2_RS = 4 * MC                # 2*MC : r^{s+1} sin(theta s)
_C2_RSN = 6 * MC               # 2*MC : negated RS
_C2_COLS = 8 * MC

DIAG_PASSES = 11               # chunk-0 D*x split into passes to bridge the
                               # exchange gap without the PE ramping down
_JORD = (0, 2, 1, 3)           # production order: re c0, im c0, re c1, im c1


def _build():
    import concourse.bass as bass  # noqa: F401
    import concourse.mybir as mybir
    import concourse.tile as tile
    from concourse import bacc

    f32 = mybir.dt.float32
    bf16 = mybir.dt.bfloat16
    ADD = mybir.AluOpType.add
    SUB = mybir.AluOpType.subtract
    MUL = mybir.AluOpType.mult

    nc = bacc.Bacc("TRN2", target_bir_lowering=False, debug=False, num_devices=NCORE)

    xd = nc.dram_tensor("xp", [128, NMC * 8 * MC], bf16, kind="ExternalInput").ap()
    bgd = nc.dram_tensor("bgp", [128, 8 * N2], bf16, kind="ExternalInput").ap()
    ctd = nc.dram_tensor("ctp", [128, 4 * D], bf16, kind="ExternalInput").ap()
    ddd = nc.dram_tensor("ddp", [128, 8 * 128], bf16, kind="ExternalInput").ap()
    ddfd = nc.dram_tensor("ddf", [128, 8 * 128], bf16, kind="ExternalInput").ap()
    cb1d = nc.dram_tensor("cb1", [128, 4 * MC], bf16, kind="ExternalInput").ap()
    cb2d = nc.dram_tensor("cb2", [128, _C2_COLS], bf16, kind="ExternalInput").ap()
    cfd = nc.dram_tensor("cf", [128, _CF_COLS], f32, kind="ExternalInput").ap()
    outd = nc.dram_tensor("outT", [128, NMC * 8 * MC], bf16,
                          kind="ExternalOutput").ap()

    with tile.TileContext(nc) as tc:
        from contextlib import ExitStack

        with ExitStack() as st:
            cpool = st.enter_context(tc.tile_pool(name="consts", bufs=1))
            xpool = st.enter_context(tc.tile_pool(name="xt", bufs=1))
            gpool = st.enter_context(tc.tile_pool(name="g", bufs=1))
            ppool = st.enter_context(tc.tile_pool(name="pb", bufs=2))
            upool = st.enter_context(tc.tile_pool(name="u", bufs=2))
            gbpool = st.enter_context(tc.tile_pool(name="gb", bufs=2))
            hpool = st.enter_context(tc.tile_pool(name="h", bufs=2))
            opool = st.enter_context(tc.tile_pool(name="o", bufs=2))
            colp = st.enter_context(tc.tile_pool(name="cols", bufs=1))
            ps = st.enter_context(tc.tile_pool(name="ps", bufs=2, space="PSUM"))
            dram = st.enter_context(tc.tile_pool(name="dram", bufs=1, space="DRAM"))

            # ---- loads; first x chunk + Bg quartered so matmuls start early ----
            bg_sb = cpool.tile([128, 8 * N2], bf16, tag="bg", name="bg")
            xt = []
            for m in range(NMC):
                xt.append(xpool.tile([128, 8 * MC], bf16, tag=f"x{m}", name=f"x{m}"))
            for q in range(4):
                nc.sync.dma_start(bg_sb[:, q * 2 * N2:(q + 1) * 2 * N2],
                                  bgd[:, q * 2 * N2:(q + 1) * 2 * N2])
                nc.sync.dma_start(xt[0][:, q * 2 * MC:(q + 1) * 2 * MC],
                                  xd[:, q * 2 * MC:(q + 1) * 2 * MC])
            cb1_sb = cpool.tile([128, 4 * MC], bf16, tag="cb1", name="cb1")
            nc.sync.dma_start(cb1_sb[:], cb1d[:, :])
            cf_sb = cpool.tile([128, _CF_COLS], f32, tag="cf", name="cf")
            nc.sync.dma_start(cf_sb[:], cfd[:, :])
            for m in range(1, NMC):
                nc.sync.dma_start(xt[m][:], xd[:, m * 8 * MC:(m + 1) * 8 * MC])
            ct_sb = cpool.tile([128, 4 * D], bf16, tag="ct", name="ct")
            nc.sync.dma_start(ct_sb[:], ctd[:, :])
            dd_sb = cpool.tile([128, 8 * 128], bf16, tag="dd", name="dd")
            nc.sync.dma_start(dd_sb[:], ddd[:, :])
            ddf_sb = cpool.tile([128, 8 * 128], bf16, tag="ddf", name="ddf")
            nc.sync.dma_start(ddf_sb[:], ddfd[:, :])
            cb2_sb = cpool.tile([128, _C2_COLS], bf16, tag="cb2", name="cb2")
            nc.sync.dma_start(cb2_sb[:], cb2d[:, :])

            def coss(c):
                return cb1_sb[:, c * MC:(c + 1) * MC]

            def sins(c):
                return cb1_sb[:, 2 * MC + c * MC:2 * MC + (c + 1) * MC]

            def cb2w(off, c):  # wide [128, MC] bf16 slice of cb2
                return cb2_sb[:, off + c * MC:off + (c + 1) * MC]

            def cfw(off, c):   # wide [128, MC] f32 slice of cf
                return cf_sb[:, off + c * MC:off + (c + 1) * MC]

            def cfc(off, i=0):  # single column
                return cf_sb[:, off + i:off + i + 1]

            g4 = []
            for j in range(4):
                g4.append(gpool.tile([128, LLOC], f32, tag=f"g{j}", name=f"g{j}"))

            jof = {("re", 0): 0, ("re", 1): 1, ("im", 0): 2, ("im", 1): 3}

            def Wcol(j, m):
                return g4[j][:, (m + 1) * MC - 1:(m + 1) * MC]

            def rotout(gb, m, pool_c=(), pool_tmp=False):
                """12 bf16 TT ops, producing h tiles in _JORD order.
                c-blocks in pool_c run fully on Pool (chunk 0 overlaps the
                exchange); pool_tmp routes just the tmp mults to Pool."""
                h4 = [None] * 4
                for c in range(2):
                    eng = nc.gpsimd if c in pool_c else nc.vector
                    teng = nc.gpsimd if (pool_tmp or c in pool_c) else eng
                    h_re = hpool.tile([128, MC], bf16, tag=f"h{c}", name=f"h{c}_{m}")
                    tmp = hpool.tile([128, MC], bf16, tag="tmpC", name=f"tC{c}_{m}")
                    teng.tensor_tensor(tmp[:], gb[2 + c][:], sins(c), MUL)
                    eng.tensor_tensor(h_re[:], gb[c][:], coss(c), MUL)
                    eng.tensor_tensor(h_re[:], h_re[:], tmp[:], SUB)
                    h4[c] = h_re
                    h_im = hpool.tile([128, MC], bf16, tag=f"h{2+c}", name=f"h{2+c}_{m}")
                    tmp2 = hpool.tile([128, MC], bf16, tag="tmpD", name=f"tD{c}_{m}")
                    teng.tensor_tensor(tmp2[:], gb[c][:], sins(c), MUL)
                    eng.tensor_tensor(h_im[:], gb[2 + c][:], coss(c), MUL)
                    eng.tensor_tensor(h_im[:], h_im[:], tmp2[:], ADD)
                    h4[2 + c] = h_im
                return h4

            # intra-core carry columns, built incrementally after each chunk
            cint = {}
            for comp in ("re", "im"):
                for c in range(2):
                    t = colp.tile([128, NMC], f32, tag=f"ci{comp}{c}",
                                  name=f"ci{comp}{c}")
                    nc.vector.memzero(t[:])
                    cint[(comp, c)] = t
            E3 = {}

            def carry_step(m, c):
                """After chunk m's c-block scans: c_{m+1} = rot(W_m + r^MC c_m),
                or E3 = W_3 + r^MC c_3 for the last chunk."""
                rmc, cmc = cfc(_CF_RMC, c), cfc(_CF_CMC, c)
                smc, smcn = cfc(_CF_SMC, c), cfc(_CF_SMCN, c)
                cre, cim = cint[("re", c)], cint[("im", c)]
                wre, wim = jof[("re", c)], jof[("im", c)]
                if m == 0:
                    ere, eim = Wcol(wre, 0), Wcol(wim, 0)
                else:
                    ere = colp.tile([128, 1], f32, tag=f"er{c}{m}",
                                    name=f"er{c}{m}")
                    nc.vector.scalar_tensor_tensor(
                        ere[:], cre[:, m:m + 1], rmc, Wcol(wre, m), MUL, ADD)
                    eim = colp.tile([128, 1], f32, tag=f"ei{c}{m}",
                                    name=f"ei{c}{m}")
                    nc.vector.scalar_tensor_tensor(
                        eim[:], cim[:, m:m + 1], rmc, Wcol(wim, m), MUL, ADD)
                    ere, eim = ere[:], eim[:]
                if m == NMC - 1:
                    E3[("re", c)] = ere
                    E3[("im", c)] = eim
                    return
                k = m + 1
                nc.vector.tensor_scalar_mul(cre[:, k:k + 1], ere, cmc)
                nc.vector.scalar_tensor_tensor(
                    cre[:, k:k + 1], eim, smcn, cre[:, k:k + 1], MUL, ADD)
                nc.vector.tensor_scalar_mul(cim[:, k:k + 1], ere, smc)
                nc.vector.scalar_tensor_tensor(
                    cim[:, k:k + 1], eim, cmc, cim[:, k:k + 1], MUL, ADD)

            # ---- phase A: in-proj -> rot-in -> per-chunk scans (init 0) ----
            for m in range(NMC):
                ms = slice(m * MC, (m + 1) * MC)
                pts = {}
                # ki rounds of 2 so chunk 0 can start on quarter-loaded x/bg
                rounds = ((0, 1), (2, 3), (4, 5), (6, 7))
                for ri, kis in enumerate(rounds):
                    for j in _JORD:
                        if ri == 0:
                            pts[j] = ps.tile([128, MC], f32, tag=f"p{j}",
                                             name=f"pa{j}_{m}")
                        for ki in kis:
                            nc.tensor.matmul(
                                pts[j][:],
                                bg_sb[:, ki * N2 + 128 * j:ki * N2 + 128 * (j + 1)],
                                xt[m][:, ki * MC:(ki + 1) * MC],
                                start=(ki == 0), stop=(ki == 7))
                pb = {}
                for j in _JORD:
                    pbt = ppool.tile([128, MC], bf16, tag=f"pb{j}", name=f"pb{j}_{m}")
                    nc.scalar.copy(pbt[:], pts[j][:])
                    pb[j] = pbt
                for c in range(2):
                    u_re = upool.tile([128, MC], bf16, tag=f"u{c}", name=f"u{c}_{m}")
                    tmp = upool.tile([128, MC], bf16, tag="tmpA", name=f"tA{c}_{m}")
                    ueng = nc.gpsimd if m == NMC - 1 else nc.vector
                    ueng.tensor_tensor(tmp[:], pb[2 + c][:], sins(c), MUL)
                    nc.vector.tensor_tensor(u_re[:], pb[c][:], coss(c), MUL)
                    nc.vector.tensor_tensor(u_re[:], u_re[:], tmp[:], ADD)
                    u_im = upool.tile([128, MC], bf16, tag=f"u{2+c}", name=f"u{2+c}_{m}")
                    tmp2 = upool.tile([128, MC], bf16, tag="tmpB", name=f"tB{c}_{m}")
                    ueng.tensor_tensor(tmp2[:], pb[c][:], sins(c), MUL)
                    nc.vector.tensor_tensor(u_im[:], pb[2 + c][:], coss(c), MUL)
                    nc.vector.tensor_tensor(u_im[:], u_im[:], tmp2[:], SUB)
                    with tc.high_priority():
                        nc.vector.tensor_tensor_scan(
                            g4[jof[("re", c)]][:, ms], cfw(_CF_RB, c), u_re[:],
                            0.0, MUL, ADD)
                        nc.vector.tensor_tensor_scan(
                            g4[jof[("im", c)]][:, ms], cfw(_CF_RB, c), u_im[:],
                            0.0, MUL, ADD)
                        carry_step(m, c)

            # ---- pre-rotate chunk 0 (emitted first: Pool/DVE run these while
            # the exchange below is in flight; emitting after in_cc would
            # head-of-line block the Pool queue) ----
            gb0 = []
            for j in range(4):
                t = gbpool.tile([128, MC], bf16, tag=f"gb{j}", name=f"gb{j}_0")
                nc.gpsimd.tensor_copy(t[:], g4[j][:, 0:MC])
                gb0.append(t)

            # ---- stage + exchange (pairwise via 16-col AllReduce) ----
            stage = colp.tile([128, 4], f32, tag="stage", name="stage")
            stage16 = colp.tile([128, 16], f32, tag="st16", name="st16")
            with tc.high_priority():
                nc.vector.tensor_copy(stage[:, 0:1], E3[("re", 0)])
                nc.vector.tensor_copy(stage[:, 1:2], E3[("re", 1)])
                nc.vector.tensor_copy(stage[:, 2:3], E3[("im", 0)])
                nc.vector.tensor_copy(stage[:, 3:4], E3[("im", 1)])
                for p in range(4):
                    nc.vector.tensor_scalar_mul(
                        stage16[:, 4 * p:4 * (p + 1)], stage[:], cfc(_CF_GM, p))
            in_cc = dram.tile([128, 16], f32, tag="incc", name="incc")
            out_cc = dram.tile([128, 16], f32, tag="outcc", name="outcc",
                               addr_space="Shared")
            nc.gpsimd.dma_start(in_cc[:], stage16[:])
            if os.environ.get("LRU_NOCC", "0") == "1":
                # collective-free variant for TimelineSim bottleneck analysis
                nc.gpsimd.dma_start(out_cc[:], in_cc[:])
            else:
                nc.gpsimd.collective_compute(
                    "AllReduce",
                    mybir.AluOpType.add,
                    replica_groups=[list(range(NCORE))],
                    ins=[in_cc.opt()],
                    outs=[out_cc.opt()],
                )
            recv16 = colp.tile([128, 16], f32, tag="recv16", name="recv16")
            nc.gpsimd.dma_start(recv16[:], out_cc[:])

            # rot-out of chunk 0 on DVE, overlapping the exchange
            h4_0 = rotout(gb0, 0, pool_c=(1,))

            # ---- receive, combine with intra carries ----
            recv = colp.tile([128, 4], f32, tag="recv", name="recv")
            nc.vector.tensor_scalar_mul(recv[:], recv16[:, 0:4], cfc(_CF_PM, 0))
            for p in range(1, 4):
                nc.vector.scalar_tensor_tensor(
                    recv[:], recv16[:, 4 * p:4 * (p + 1)], cfc(_CF_PM, p),
                    recv[:], MUL, ADD)
            Hre = {0: recv[:, 0:1], 1: recv[:, 1:2]}
            Him = {0: recv[:, 2:3], 1: recv[:, 3:4]}
            ctot = {}
            for c in range(2):
                rfc = cf_sb[:, _CF_RFC + 4 * c:_CF_RFC + 4 * (c + 1)]
                rfs = cf_sb[:, _CF_RFS + 4 * c:_CF_RFS + 4 * (c + 1)]
                rfsn = cf_sb[:, _CF_RFSN + 4 * c:_CF_RFSN + 4 * (c + 1)]
                tre = colp.tile([128, 4], f32, tag=f"ct_re{c}", name=f"ct_re{c}")
                nc.vector.scalar_tensor_tensor(
                    tre[:], rfc, Hre[c], cint[("re", c)][:], MUL, ADD)
                nc.vector.scalar_tensor_tensor(
                    tre[:], rfsn, Him[c], tre[:], MUL, ADD)
                tim = colp.tile([128, 4], f32, tag=f"ct_im{c}", name=f"ct_im{c}")
                nc.vector.scalar_tensor_tensor(
                    tim[:], rfs, Hre[c], cint[("im", c)][:], MUL, ADD)
                nc.vector.scalar_tensor_tensor(
                    tim[:], rfc, Him[c], tim[:], MUL, ADD)
                ctot[("re", c)] = tre
                ctot[("im", c)] = tim

            # ---- chunk 0 carry fix in h-space: h += e^{i th s} r^{s+1} c ----
            # emitted in _JORD so the first CT round can start after 2 ops;
            # chunk 1's w-space fix is wedged between the halves so its
            # longer fix->copy->rotout chain starts early
            def wfix(m):
                for c in range(2):
                    ms_ = slice(m * MC, (m + 1) * MC)
                    nc.vector.scalar_tensor_tensor(
                        g4[jof[("re", c)]][:, ms_], cb2w(_C2_RPOW, c),
                        ctot[("re", c)][:, m:m + 1],
                        g4[jof[("re", c)]][:, ms_], MUL, ADD)
                    nc.vector.scalar_tensor_tensor(
                        g4[jof[("im", c)]][:, ms_], cb2w(_C2_RPOW, c),
                        ctot[("im", c)][:, m:m + 1],
                        g4[jof[("im", c)]][:, ms_], MUL, ADD)

            for c in range(2):
                h_re, h_im = h4_0[c], h4_0[2 + c]
                nc.vector.scalar_tensor_tensor(
                    h_re[:], cb2w(_C2_RC, c), ctot[("re", c)][:, 0:1],
                    h_re[:], MUL, ADD)
                nc.vector.scalar_tensor_tensor(
                    h_re[:], cb2w(_C2_RSN, c), ctot[("im", c)][:, 0:1],
                    h_re[:], MUL, ADD)
                nc.vector.scalar_tensor_tensor(
                    h_im[:], cb2w(_C2_RS, c), ctot[("re", c)][:, 0:1],
                    h_im[:], MUL, ADD)
                nc.vector.scalar_tensor_tensor(
                    h_im[:], cb2w(_C2_RC, c), ctot[("im", c)][:, 0:1],
                    h_im[:], MUL, ADD)
                if c == 0:
                    wfix(1)

            # ---- phase C: per chunk fix -> rot-out -> out-proj -> store ----
            for m in range(NMC):
                ms = slice(m * MC, (m + 1) * MC)
                if m == 0:
                    h4 = h4_0
                else:
                    if m > 1:   # m == 1 fixed early, above
                        wfix(m)
                    gb = []
                    for j in range(4):
                        t = gbpool.tile([128, MC], bf16, tag=f"gb{j}",
                                        name=f"gb{j}_{m}")
                        nc.scalar.copy(t[:], g4[j][:, ms])
                        gb.append(t)
                    h4 = rotout(gb, m, pool_tmp=True)
                osb = opool.tile([128, 8 * MC], bf16, tag="osb", name=f"osb{m}")
                pts = []
                passes = DIAG_PASSES if m == 0 else 1
                dsb = ddf_sb if m == 0 else dd_sb
                for di in range(8):
                    pt = ps.tile([128, MC], f32, tag=f"p{di % 4}", name=f"o{di}_{m}")
                    nc.tensor.matmul(
                        pt[:], dsb[:, di * 128:(di + 1) * 128],
                        xt[m][:, di * MC:(di + 1) * MC],
                        start=True, stop=False)
                    pts.append(pt)
                for _ in range(passes - 1):
                    for di in range(8):
                        nc.tensor.matmul(
                            pts[di][:], dsb[:, di * 128:(di + 1) * 128],
                            xt[m][:, di * MC:(di + 1) * MC],
                            start=False, stop=False)
                last = m == NMC - 1
                for half in (range(0, 4), range(4, 8)):
                    if last and half.start == 4:
                        # drain the tail: di-major so copies/stores start early
                        for di in half:
                            for tt in _JORD:
                                nc.tensor.matmul(
                                    pts[di][:],
                                    ct_sb[:, tt * D + di * 128:
                                          tt * D + (di + 1) * 128],
                                    h4[tt][:],
                                    start=False, stop=(tt == 3))
                            nc.scalar.copy(
                                osb[:, di * MC:(di + 1) * MC], pts[di][:])
                        for di in half:
                            nc.sync.dma_start(
                                outd[:, (m * 8 + di) * MC:(m * 8 + di + 1) * MC],
                                osb[:, di * MC:(di + 1) * MC])
                        continue
                    for tt in _JORD:
                        for di in half:
                            nc.tensor.matmul(
                                pts[di][:],
                                ct_sb[:, tt * D + di * 128:tt * D + (di + 1) * 128],
                                h4[tt][:],
                                start=False, stop=(tt == 3))
                    for di in half:
                        nc.scalar.copy(osb[:, di * MC:(di + 1) * MC], pts[di][:])
                    lo, hi = half.start * MC, half.stop * MC
                    nc.sync.dma_start(
                        outd[:, m * 8 * MC + lo:m * 8 * MC + hi], osb[:, lo:hi])

    nc.compile()
    return nc


def _prep(inputs):
    """Host-side parameter prep + sharding. Returns per-core input maps."""
    import ml_dtypes

    bf = ml_dtypes.bfloat16
    x = np.asarray(inputs["input_sequence"], np.float32)
    nu_log = np.asarray(inputs["nu_log"], np.float32)
    theta_log = np.asarray(inputs["theta_log"], np.float32)
    B_re = np.asarray(inputs["B_re"], np.float32)
    B_im = np.asarray(inputs["B_im"], np.float32)
    C_re = np.asarray(inputs["C_re"], np.float32)
    C_im = np.asarray(inputs["C_im"], np.float32)
    Dv = np.asarray(inputs["D"], np.float32)

    r32 = np.exp(-np.exp(nu_log, dtype=np.float32), dtype=np.float32)
    th = np.exp(theta_log, dtype=np.float32).astype(np.float64)
    r64 = r32.astype(np.float64)
    gamma = np.sqrt((1.0 - r32 * r32).astype(np.float32))

    bg = np.concatenate([(gamma[:, None] * B_re).T, (gamma[:, None] * B_im).T],
                        axis=1)                        # [D, 512]
    bgp = np.ascontiguousarray(
        bg.reshape(8, 128, N2).transpose(1, 0, 2).reshape(128, 8 * N2)).astype(bf)
    ct = np.concatenate([C_re.T, -C_im.T], axis=0)     # [512, D]
    ctp = np.ascontiguousarray(
        ct.reshape(4, 128, D).transpose(1, 0, 2).reshape(128, 4 * D)).astype(bf)
    ddp = np.zeros((128, 8 * 128), np.float32)
    for di in range(8):
        idx = np.arange(128)
        ddp[idx, di * 128 + idx] = Dv[di * 128 + idx]
    ddf = (ddp / DIAG_PASSES).astype(bf)
    ddp = ddp.astype(bf)

    s = np.arange(MC, dtype=np.float64)
    ang = th[:, None] * s[None, :]                     # [N, MC]
    cosv = np.cos(ang)
    sinv = np.sin(ang)
    cb1 = np.zeros((128, 4 * MC), np.float64)
    for c in range(2):
        rows = slice(128 * c, 128 * (c + 1))
        cb1[:, c * MC:(c + 1) * MC] = cosv[rows]
        cb1[:, 2 * MC + c * MC:2 * MC + (c + 1) * MC] = sinv[rows]
    cb1 = cb1.astype(bf)

    rpow = r64[:, None] ** (s[None, :] + 1.0)          # [N, MC]
    rc = rpow * cosv
    rs = rpow * sinv
    cb2 = np.zeros((128, _C2_COLS), np.float64)
    for c in range(2):
        rows = slice(128 * c, 128 * (c + 1))
        cb2[:, _C2_RPOW + c * MC:_C2_RPOW + (c + 1) * MC] = rpow[rows]
        cb2[:, _C2_RC + c * MC:_C2_RC + (c + 1) * MC] = rc[rows]
        cb2[:, _C2_RS + c * MC:_C2_RS + (c + 1) * MC] = rs[rows]
        cb2[:, _C2_RSN + c * MC:_C2_RSN + (c + 1) * MC] = -rs[rows]
    cb2 = cb2.astype(bf)

    rmc = r64 ** MC
    cmc = np.cos(th * MC)
    smc = np.sin(th * MC)
    mm = np.arange(NMC, dtype=np.float64)
    rfm = r64[:, None] ** (mm[None, :] * MC)           # [N, 4]
    angm = th[:, None] * ((mm[None, :] + 1.0) * MC)
    rfc = rfm * np.cos(angm)
    rfs = rfm * np.sin(angm)

    cf_base = np.zeros((128, _CF_COLS), np.float64)
    for c in range(2):
        rows = slice(128 * c, 128 * (c + 1))
        cf_base[:, _CF_RB + c * MC:_CF_RB + (c + 1) * MC] = r64[rows, None]
        cf_base[:, _CF_RMC + c] = rmc[rows]
        cf_base[:, _CF_CMC + c] = cmc[rows]
        cf_base[:, _CF_SMC + c] = smc[rows]
        cf_base[:, _CF_SMCN + c] = -smc[rows]

    in_maps = []
    for core in range(NCORE):
        b, h = core // 2, core % 2
        cf = cf_base.copy()
        if h == 1:   # only second-half cores consume the received carry
            for c in range(2):
                rows = slice(128 * c, 128 * (c + 1))
                cf[:, _CF_RFC + 4 * c:_CF_RFC + 4 * (c + 1)] = rfc[rows]
                cf[:, _CF_RFS + 4 * c:_CF_RFS + 4 * (c + 1)] = rfs[rows]
                cf[:, _CF_RFSN + 4 * c:_CF_RFSN + 4 * (c + 1)] = -rfs[rows]
        if h == 0:   # first-half cores contribute their end state to group b
            cf[:, _CF_GM + b] = 1.0
        cf[:, _CF_PM + b] = 1.0
        cf = cf.astype(np.float32)

        xs = x[b, h * LLOC:(h + 1) * LLOC, :]          # [LLOC, D]
        # device layout: xp[p, m*8*MC + ki*MC + s] = x^T[ki*128+p, m*MC+s]
        xp = np.ascontiguousarray(
            xs.T.reshape(8, 128, NMC, MC).transpose(1, 2, 0, 3).reshape(
                128, NMC * 8 * MC)).astype(bf)
        in_maps.append({
            "xp": xp, "bgp": bgp, "ctp": ctp, "ddp": ddp, "ddf": ddf,
            "cb1": cb1, "cb2": cb2, "cf": cf,
        })
    return in_maps


def kernel(**inputs) -> np.ndarray:
    global LAST_RESULTS
    from concourse.bass_utils import run_bass_kernel_spmd

    if "nc" not in _CACHE:
        _CACHE["nc"] = _build()
    nc = _CACHE["nc"]

    in_maps = _prep(inputs)
    trace = os.environ.get("LRU_TRACE", "0") == "1"
    res = run_bass_kernel_spmd(
        nc, in_maps, core_ids=list(range(NCORE)), trace=trace,
        trace_cores=list(range(NCORE)) if trace else None,
        stitch_traces=trace,
    )
    LAST_RESULTS = res

    out = np.empty((B, L, D), np.float32)
    for core in range(NCORE):
        b, h = core // 2, core % 2
        o = np.asarray(res.results[core]["outT"]).astype(np.float32)
        # o[p, m*8*MC + di*MC + s] = y^T[di*128+p, m*MC+s]
        yT = o.reshape(128, NMC, 8, MC).transpose(2, 0, 1, 3).reshape(D, LLOC)
        out[b, h * LLOC:(h + 1) * LLOC, :] = yT.T
    return out


# revision 22
# speedup vs baseline: 1.0210x; 1.0210x over previous
"""LRU forward on 8 Trainium2 NeuronCores.

Sharding: 8 shards = 4 batches x 2 sequence halves (L_local = 2048).
Per-core dataflow is fully transposed (channels on SBUF partitions, time on
the free dim), bf16 matmul operands, and a CHUNK-LOCAL rotating frame:

  in proj   p = Bg^T @ x            (bf16 matmuls, fp32 PSUM)
  rot-in    v[s] = e^{-i theta s} p[s]   per 512-chunk local time s
            (bf16 DVE tensor_tensor, 2x perf mode)
  scan      w = scan(r, v)          4 real per-lane hw scans, init 0 per chunk
  carry     column-only recurrence c_{m+1} = e^{i theta MC}(W_m + r^MC c_m),
            cross-core end-state exchanged via a 16-col AllReduce
  fix       chunk 0: h-space  h += e^{i theta s} r^{s+1} c
            chunks 1-3: w-space  w += r^{s+1} c
  rot-out   h[s] = e^{+i theta s} w[s]  (bf16)
  out proj  y^T = CT^T @ h + diag(D) blocks @ x^T   (bf16 matmuls)

The chunk-local frame needs only [256, 512] cos/sin tables (vs [256, 2048]
twice), loaded once. All big DMAs are batched bf16 transfers, with the
first x chunk and Bg split in halves so the tensor engine starts early.
"""

import os

import numpy as np

B, L, D, N = 4, 4096, 1024, 256
NCORE = 8
LLOC = L // 2          # per-core sequence length
MC = 512               # time chunk
NMC = LLOC // MC       # 4 chunks
N2 = 2 * N             # stacked re|im channels

_CACHE = {}
LAST_RESULTS = None    # test.py reads exec_time_ns from here

# cf (f32 const pack) column offsets
_CF_RB = 0                     # 2*MC : r broadcast, per c-block
_CF_RMC = 2 * MC               # +c   : r^{MC}
_CF_CMC = 2 * MC + 2           # +c   : cos(theta MC)
_CF_SMC = 2 * MC + 4           # +c   : sin(theta MC)
_CF_SMCN = 2 * MC + 6          # +c   : -sin(theta MC)
_CF_RFC = 2 * MC + 8           # +4c+m: r^{mMC} cos(theta (m+1) MC)  (0 if h=0)
_CF_RFS = 2 * MC + 16          # +4c+m: r^{mMC} sin(theta (m+1) MC)
_CF_RFSN = 2 * MC + 24         # +4c+m: negated RFS
_CF_GM = 2 * MC + 32           # +p   : contribute mask
_CF_PM = 2 * MC + 36           # +p   : select mask
_CF_COLS = 2 * MC + 40

# cb2 (bf16 const pack) column offsets
_C2_RPOW = 0                   # 2*MC : r^{s+1}
_C2_RC = 2 * MC                # 2*MC : r^{s+1} cos(theta s)
_C2_RS = 4 * MC                # 2*MC : r^{s+1} sin(theta s)
_C2_RSN = 6 * MC               # 2*MC : negated RS
_C2_COLS = 8 * MC

DIAG_PASSES = 9               # chunk-0 D*x split into passes to bridge the
                               # exchange gap without the PE ramping down
_JORD = (0, 2, 1, 3)           # production order: re c0, im c0, re c1, im c1


def _build():
    import concourse.bass as bass  # noqa: F401
    import concourse.mybir as mybir
    import concourse.tile as tile
    from concourse import bacc

    f32 = mybir.dt.float32
    bf16 = mybir.dt.bfloat16
    ADD = mybir.AluOpType.add
    SUB = mybir.AluOpType.subtract
    MUL = mybir.AluOpType.mult

    nc = bacc.Bacc("TRN2", target_bir_lowering=False, debug=False, num_devices=NCORE)

    xd = nc.dram_tensor("xp", [128, NMC * 8 * MC], bf16, kind="ExternalInput").ap()
    bgd = nc.dram_tensor("bgp", [128, 8 * N2], bf16, kind="ExternalInput").ap()
    ctd = nc.dram_tensor("ctp", [128, 4 * D], bf16, kind="ExternalInput").ap()
    ddd = nc.dram_tensor("ddp", [128, 8 * 128], bf16, kind="ExternalInput").ap()
    ddfd = nc.dram_tensor("ddf", [128, 8 * 128], bf16, kind="ExternalInput").ap()
    cb1d = nc.dram_tensor("cb1", [128, 4 * MC], bf16, kind="ExternalInput").ap()
    cb2d = nc.dram_tensor("cb2", [128, _C2_COLS], bf16, kind="ExternalInput").ap()
    cfd = nc.dram_tensor("cf", [128, _CF_COLS], f32, kind="ExternalInput").ap()
    outd = nc.dram_tensor("outT", [128, NMC * 8 * MC], bf16,
                          kind="ExternalOutput").ap()

    with tile.TileContext(nc) as tc:
        from contextlib import ExitStack

        with ExitStack() as st:
            cpool = st.enter_context(tc.tile_pool(name="consts", bufs=1))
            xpool = st.enter_context(tc.tile_pool(name="xt", bufs=1))
            gpool = st.enter_context(tc.tile_pool(name="g", bufs=1))
            ppool = st.enter_context(tc.tile_pool(name="pb", bufs=2))
            upool = st.enter_context(tc.tile_pool(name="u", bufs=2))
            gbpool = st.enter_context(tc.tile_pool(name="gb", bufs=2))
            hpool = st.enter_context(tc.tile_pool(name="h", bufs=2))
            opool = st.enter_context(tc.tile_pool(name="o", bufs=2))
            colp = st.enter_context(tc.tile_pool(name="cols", bufs=1))
            ps = st.enter_context(tc.tile_pool(name="ps", bufs=2, space="PSUM"))
            dram = st.enter_context(tc.tile_pool(name="dram", bufs=1, space="DRAM"))

            # ---- loads; first x chunk + Bg quartered so matmuls start early ----
            bg_sb = cpool.tile([128, 8 * N2], bf16, tag="bg", name="bg")
            xt = []
            for m in range(NMC):
                xt.append(xpool.tile([128, 8 * MC], bf16, tag=f"x{m}", name=f"x{m}"))
            for q in range(4):
                nc.sync.dma_start(bg_sb[:, q * 2 * N2:(q + 1) * 2 * N2],
                                  bgd[:, q * 2 * N2:(q + 1) * 2 * N2])
                nc.sync.dma_start(xt[0][:, q * 2 * MC:(q + 1) * 2 * MC],
                                  xd[:, q * 2 * MC:(q + 1) * 2 * MC])
            cb1_sb = cpool.tile([128, 4 * MC], bf16, tag="cb1", name="cb1")
            nc.sync.dma_start(cb1_sb[:], cb1d[:, :])
            cf_sb = cpool.tile([128, _CF_COLS], f32, tag="cf", name="cf")
            nc.sync.dma_start(cf_sb[:], cfd[:, :])
            for m in range(1, NMC):
                nc.sync.dma_start(xt[m][:], xd[:, m * 8 * MC:(m + 1) * 8 * MC])
            ct_sb = cpool.tile([128, 4 * D], bf16, tag="ct", name="ct")
            nc.sync.dma_start(ct_sb[:], ctd[:, :])
            dd_sb = cpool.tile([128, 8 * 128], bf16, tag="dd", name="dd")
            nc.sync.dma_start(dd_sb[:], ddd[:, :])
            ddf_sb = cpool.tile([128, 8 * 128], bf16, tag="ddf", name="ddf")
            nc.sync.dma_start(ddf_sb[:], ddfd[:, :])
            cb2_sb = cpool.tile([128, _C2_COLS], bf16, tag="cb2", name="cb2")
            nc.sync.dma_start(cb2_sb[:], cb2d[:, :])

            def coss(c):
                return cb1_sb[:, c * MC:(c + 1) * MC]

            def sins(c):
                return cb1_sb[:, 2 * MC + c * MC:2 * MC + (c + 1) * MC]

            def cb2w(off, c):  # wide [128, MC] bf16 slice of cb2
                return cb2_sb[:, off + c * MC:off + (c + 1) * MC]

            def cfw(off, c):   # wide [128, MC] f32 slice of cf
                return cf_sb[:, off + c * MC:off + (c + 1) * MC]

            def cfc(off, i=0):  # single column
                return cf_sb[:, off + i:off + i + 1]

            g4 = []
            for j in range(4):
                g4.append(gpool.tile([128, LLOC], f32, tag=f"g{j}", name=f"g{j}"))

            jof = {("re", 0): 0, ("re", 1): 1, ("im", 0): 2, ("im", 1): 3}

            def Wcol(j, m):
                return g4[j][:, (m + 1) * MC - 1:(m + 1) * MC]

            def rotout(gb, m, pool_c=(), pool_tmp=False):
                """12 bf16 TT ops, producing h tiles in _JORD order.
                c-blocks in pool_c run fully on Pool (chunk 0 overlaps the
                exchange); pool_tmp routes just the tmp mults to Pool."""
                h4 = [None] * 4
                for c in range(2):
                    eng = nc.gpsimd if c in pool_c else nc.vector
                    teng = nc.gpsimd if (pool_tmp or c in pool_c) else eng
                    h_re = hpool.tile([128, MC], bf16, tag=f"h{c}", name=f"h{c}_{m}")
                    tmp = hpool.tile([128, MC], bf16, tag="tmpC", name=f"tC{c}_{m}")
                    teng.tensor_tensor(tmp[:], gb[2 + c][:], sins(c), MUL)
                    eng.tensor_tensor(h_re[:], gb[c][:], coss(c), MUL)
                    eng.tensor_tensor(h_re[:], h_re[:], tmp[:], SUB)
                    h4[c] = h_re
                    h_im = hpool.tile([128, MC], bf16, tag=f"h{2+c}", name=f"h{2+c}_{m}")
                    tmp2 = hpool.tile([128, MC], bf16, tag="tmpD", name=f"tD{c}_{m}")
                    teng.tensor_tensor(tmp2[:], gb[c][:], sins(c), MUL)
                    eng.tensor_tensor(h_im[:], gb[2 + c][:], coss(c), MUL)
                    eng.tensor_tensor(h_im[:], h_im[:], tmp2[:], ADD)
                    h4[2 + c] = h_im
                return h4

            # intra-core carry columns, built incrementally after each chunk
            cint = {}
            for comp in ("re", "im"):
                for c in range(2):
                    t = colp.tile([128, NMC], f32, tag=f"ci{comp}{c}",
                                  name=f"ci{comp}{c}")
                    nc.vector.memzero(t[:])
                    cint[(comp, c)] = t
            E3 = {}

            def carry_step(m, c):
                """After chunk m's c-block scans: c_{m+1} = rot(W_m + r^MC c_m),
                or E3 = W_3 + r^MC c_3 for the last chunk."""
                rmc, cmc = cfc(_CF_RMC, c), cfc(_CF_CMC, c)
                smc, smcn = cfc(_CF_SMC, c), cfc(_CF_SMCN, c)
                cre, cim = cint[("re", c)], cint[("im", c)]
                wre, wim = jof[("re", c)], jof[("im", c)]
                if m == 0:
                    ere, eim = Wcol(wre, 0), Wcol(wim, 0)
                else:
                    ere = colp.tile([128, 1], f32, tag=f"er{c}{m}",
                                    name=f"er{c}{m}")
                    nc.vector.scalar_tensor_tensor(
                        ere[:], cre[:, m:m + 1], rmc, Wcol(wre, m), MUL, ADD)
                    eim = colp.tile([128, 1], f32, tag=f"ei{c}{m}",
                                    name=f"ei{c}{m}")
                    nc.vector.scalar_tensor_tensor(
                        eim[:], cim[:, m:m + 1], rmc, Wcol(wim, m), MUL, ADD)
                    ere, eim = ere[:], eim[:]
                if m == NMC - 1:
                    E3[("re", c)] = ere
                    E3[("im", c)] = eim
                    return
                k = m + 1
                nc.vector.tensor_scalar_mul(cre[:, k:k + 1], ere, cmc)
                nc.vector.scalar_tensor_tensor(
                    cre[:, k:k + 1], eim, smcn, cre[:, k:k + 1], MUL, ADD)
                nc.vector.tensor_scalar_mul(cim[:, k:k + 1], ere, smc)
                nc.vector.scalar_tensor_tensor(
                    cim[:, k:k + 1], eim, cmc, cim[:, k:k + 1], MUL, ADD)

            # ---- phase A: in-proj -> rot-in -> per-chunk scans (init 0) ----
            for m in range(NMC):
                ms = slice(m * MC, (m + 1) * MC)
                pts = {}
                # ki rounds of 2 so chunk 0 can start on quarter-loaded x/bg
                rounds = ((0, 1), (2, 3), (4, 5), (6, 7))
                for ri, kis in enumerate(rounds):
                    for j in _JORD:
                        if ri == 0:
                            pts[j] = ps.tile([128, MC], f32, tag=f"p{j}",
                                             name=f"pa{j}_{m}")
                        for ki in kis:
                            nc.tensor.matmul(
                                pts[j][:],
                                bg_sb[:, ki * N2 + 128 * j:ki * N2 + 128 * (j + 1)],
                                xt[m][:, ki * MC:(ki + 1) * MC],
                                start=(ki == 0), stop=(ki == 7))
                pb = {}
                for j in _JORD:
                    pbt = ppool.tile([128, MC], bf16, tag=f"pb{j}", name=f"pb{j}_{m}")
                    nc.scalar.copy(pbt[:], pts[j][:])
                    pb[j] = pbt
                for c in range(2):
                    u_re = upool.tile([128, MC], bf16, tag=f"u{c}", name=f"u{c}_{m}")
                    tmp = upool.tile([128, MC], bf16, tag="tmpA", name=f"tA{c}_{m}")
                    ueng = nc.gpsimd if m == NMC - 1 else nc.vector
                    ueng.tensor_tensor(tmp[:], pb[2 + c][:], sins(c), MUL)
                    nc.vector.tensor_tensor(u_re[:], pb[c][:], coss(c), MUL)
                    nc.vector.tensor_tensor(u_re[:], u_re[:], tmp[:], ADD)
                    u_im = upool.tile([128, MC], bf16, tag=f"u{2+c}", name=f"u{2+c}_{m}")
                    tmp2 = upool.tile([128, MC], bf16, tag="tmpB", name=f"tB{c}_{m}")
                    ueng.tensor_tensor(tmp2[:], pb[c][:], sins(c), MUL)
                    nc.vector.tensor_tensor(u_im[:], pb[2 + c][:], coss(c), MUL)
                    nc.vector.tensor_tensor(u_im[:], u_im[:], tmp2[:], SUB)
                    with tc.high_priority():
                        nc.vector.tensor_tensor_scan(
                            g4[jof[("re", c)]][:, ms], cfw(_CF_RB, c), u_re[:],
                            0.0, MUL, ADD)
                        nc.vector.tensor_tensor_scan(
                            g4[jof[("im", c)]][:, ms], cfw(_CF_RB, c), u_im[:],
                            0.0, MUL, ADD)
                        carry_step(m, c)

            # ---- pre-rotate chunk 0 (emitted first: Pool/DVE run these while
            # the exchange below is in flight; emitting after in_cc would
            # head-of-line block the Pool queue) ----
            gb0 = []
            for j in range(4):
                t = gbpool.tile([128, MC], bf16, tag=f"gb{j}", name=f"gb{j}_0")
                nc.gpsimd.tensor_copy(t[:], g4[j][:, 0:MC])
                gb0.append(t)

            # ---- stage + exchange (pairwise via 16-col AllReduce) ----
            stage = colp.tile([128, 4], f32, tag="stage", name="stage")
            stage16 = colp.tile([128, 16], f32, tag="st16", name="st16")
            with tc.high_priority():
                nc.vector.tensor_copy(stage[:, 0:1], E3[("re", 0)])
                nc.vector.tensor_copy(stage[:, 1:2], E3[("re", 1)])
                nc.vector.tensor_copy(stage[:, 2:3], E3[("im", 0)])
                nc.vector.tensor_copy(stage[:, 3:4], E3[("im", 1)])
                for p in range(4):
                    nc.vector.tensor_scalar_mul(
                        stage16[:, 4 * p:4 * (p + 1)], stage[:], cfc(_CF_GM, p))
            in_cc = dram.tile([128, 16], f32, tag="incc", name="incc")
            out_cc = dram.tile([128, 16], f32, tag="outcc", name="outcc",
                               addr_space="Shared")
            nc.gpsimd.dma_start(in_cc[:], stage16[:])
            if os.environ.get("LRU_NOCC", "0") == "1":
                # collective-free variant for TimelineSim bottleneck analysis
                nc.gpsimd.dma_start(out_cc[:], in_cc[:])
            else:
                nc.gpsimd.collective_compute(
                    "AllReduce",
                    mybir.AluOpType.add,
                    replica_groups=[list(range(NCORE))],
                    ins=[in_cc.opt()],
                    outs=[out_cc.opt()],
                )
            recv16 = colp.tile([128, 16], f32, tag="recv16", name="recv16")
            nc.gpsimd.dma_start(recv16[:], out_cc[:])

            # rot-out of chunk 0 on DVE, overlapping the exchange
            h4_0 = rotout(gb0, 0, pool_c=(1,))

            # ---- receive, combine with intra carries ----
            recv = colp.tile([128, 4], f32, tag="recv", name="recv")
            nc.vector.tensor_scalar_mul(recv[:], recv16[:, 0:4], cfc(_CF_PM, 0))
            for p in range(1, 4):
                nc.vector.scalar_tensor_tensor(
                    recv[:], recv16[:, 4 * p:4 * (p + 1)], cfc(_CF_PM, p),
                    recv[:], MUL, ADD)
            Hre = {0: recv[:, 0:1], 1: recv[:, 1:2]}
            Him = {0: recv[:, 2:3], 1: recv[:, 3:4]}
            ctot = {}
            for c in range(2):
                rfc = cf_sb[:, _CF_RFC + 4 * c:_CF_RFC + 4 * (c + 1)]
                rfs = cf_sb[:, _CF_RFS + 4 * c:_CF_RFS + 4 * (c + 1)]
                rfsn = cf_sb[:, _CF_RFSN + 4 * c:_CF_RFSN + 4 * (c + 1)]
                tre = colp.tile([128, 4], f32, tag=f"ct_re{c}", name=f"ct_re{c}")
                nc.vector.scalar_tensor_tensor(
                    tre[:], rfc, Hre[c], cint[("re", c)][:], MUL, ADD)
                nc.vector.scalar_tensor_tensor(
                    tre[:], rfsn, Him[c], tre[:], MUL, ADD)
                tim = colp.tile([128, 4], f32, tag=f"ct_im{c}", name=f"ct_im{c}")
                nc.vector.scalar_tensor_tensor(
                    tim[:], rfs, Hre[c], cint[("im", c)][:], MUL, ADD)
                nc.vector.scalar_tensor_tensor(
                    tim[:], rfc, Him[c], tim[:], MUL, ADD)
                ctot[("re", c)] = tre
                ctot[("im", c)] = tim

            # ---- chunk 0 carry fix in h-space: h += e^{i th s} r^{s+1} c ----
            # emitted in _JORD so the first CT round can start after 2 ops;
            # chunk 1's w-space fix is wedged between the halves so its
            # longer fix->copy->rotout chain starts early
            def wfix(m):
                for c in range(2):
                    ms_ = slice(m * MC, (m + 1) * MC)
                    nc.vector.scalar_tensor_tensor(
                        g4[jof[("re", c)]][:, ms_], cb2w(_C2_RPOW, c),
                        ctot[("re", c)][:, m:m + 1],
                        g4[jof[("re", c)]][:, ms_], MUL, ADD)
                    nc.vector.scalar_tensor_tensor(
                        g4[jof[("im", c)]][:, ms_], cb2w(_C2_RPOW, c),
                        ctot[("im", c)][:, m:m + 1],
                        g4[jof[("im", c)]][:, ms_], MUL, ADD)

            for c in range(2):
                h_re, h_im = h4_0[c], h4_0[2 + c]
                nc.vector.scalar_tensor_tensor(
                    h_re[:], cb2w(_C2_RC, c), ctot[("re", c)][:, 0:1],
                    h_re[:], MUL, ADD)
                nc.vector.scalar_tensor_tensor(
                    h_re[:], cb2w(_C2_RSN, c), ctot[("im", c)][:, 0:1],
                    h_re[:], MUL, ADD)
                nc.vector.scalar_tensor_tensor(
                    h_im[:], cb2w(_C2_RS, c), ctot[("re", c)][:, 0:1],
                    h_im[:], MUL, ADD)
                nc.vector.scalar_tensor_tensor(
                    h_im[:], cb2w(_C2_RC, c), ctot[("im", c)][:, 0:1],
                    h_im[:], MUL, ADD)
                if c == 0:
                    wfix(1)

            # ---- phase C: per chunk fix -> rot-out -> out-proj -> store ----
            for m in range(NMC):
                ms = slice(m * MC, (m + 1) * MC)
                if m == 0:
                    h4 = h4_0
                else:
                    if m > 1:   # m == 1 fixed early, above
                        wfix(m)
                    gb = []
                    for j in range(4):
                        t = gbpool.tile([128, MC], bf16, tag=f"gb{j}",
                                        name=f"gb{j}_{m}")
                        nc.scalar.copy(t[:], g4[j][:, ms])
                        gb.append(t)
                    h4 = rotout(gb, m, pool_tmp=True)
                osb = opool.tile([128, 8 * MC], bf16, tag="osb", name=f"osb{m}")
                pts = []
                passes = DIAG_PASSES if m == 0 else 1
                dsb = ddf_sb if m == 0 else dd_sb
                for di in range(8):
                    pt = ps.tile([128, MC], f32, tag=f"p{di % 4}", name=f"o{di}_{m}")
                    nc.tensor.matmul(
                        pt[:], dsb[:, di * 128:(di + 1) * 128],
                        xt[m][:, di * MC:(di + 1) * MC],
                        start=True, stop=False)
                    pts.append(pt)
                for _ in range(passes - 1):
                    for di in range(8):
                        nc.tensor.matmul(
                            pts[di][:], dsb[:, di * 128:(di + 1) * 128],
                            xt[m][:, di * MC:(di + 1) * MC],
                            start=False, stop=False)
                last = m == NMC - 1
                for half in (range(0, 4), range(4, 8)):
                    if last and half.start == 4:
                        # drain the tail: di-major so copies/stores start early
                        for di in half:
                            for tt in _JORD:
                                nc.tensor.matmul(
                                    pts[di][:],
                                    ct_sb[:, tt * D + di * 128:
                                          tt * D + (di + 1) * 128],
                                    h4[tt][:],
                                    start=False, stop=(tt == 3))
                            nc.scalar.copy(
                                osb[:, di * MC:(di + 1) * MC], pts[di][:])
                        for di in half:
                            nc.sync.dma_start(
                                outd[:, (m * 8 + di) * MC:(m * 8 + di + 1) * MC],
                                osb[:, di * MC:(di + 1) * MC])
                        continue
                    for tt in _JORD:
                        for di in half:
                            nc.tensor.matmul(
                                pts[di][:],
                                ct_sb[:, tt * D + di * 128:tt * D + (di + 1) * 128],
                                h4[tt][:],
                                start=False, stop=(tt == 3))
                    for di in half:
                        nc.scalar.copy(osb[:, di * MC:(di + 1) * MC], pts[di][:])
                    lo, hi = half.start * MC, half.stop * MC
                    nc.sync.dma_start(
                        outd[:, m * 8 * MC + lo:m * 8 * MC + hi], osb[:, lo:hi])

    nc.compile()
    return nc


def _prep(inputs):
    """Host-side parameter prep + sharding. Returns per-core input maps."""
    import ml_dtypes

    bf = ml_dtypes.bfloat16
    x = np.asarray(inputs["input_sequence"], np.float32)
    nu_log = np.asarray(inputs["nu_log"], np.float32)
    theta_log = np.asarray(inputs["theta_log"], np.float32)
    B_re = np.asarray(inputs["B_re"], np.float32)
    B_im = np.asarray(inputs["B_im"], np.float32)
    C_re = np.asarray(inputs["C_re"], np.float32)
    C_im = np.asarray(inputs["C_im"], np.float32)
    Dv = np.asarray(inputs["D"], np.float32)

    r32 = np.exp(-np.exp(nu_log, dtype=np.float32), dtype=np.float32)
    th = np.exp(theta_log, dtype=np.float32).astype(np.float64)
    r64 = r32.astype(np.float64)
    gamma = np.sqrt((1.0 - r32 * r32).astype(np.float32))

    bg = np.concatenate([(gamma[:, None] * B_re).T, (gamma[:, None] * B_im).T],
                        axis=1)                        # [D, 512]
    bgp = np.ascontiguousarray(
        bg.reshape(8, 128, N2).transpose(1, 0, 2).reshape(128, 8 * N2)).astype(bf)
    ct = np.concatenate([C_re.T, -C_im.T], axis=0)     # [512, D]
    ctp = np.ascontiguousarray(
        ct.reshape(4, 128, D).transpose(1, 0, 2).reshape(128, 4 * D)).astype(bf)
    ddp = np.zeros((128, 8 * 128), np.float32)
    for di in range(8):
        idx = np.arange(128)
        ddp[idx, di * 128 + idx] = Dv[di * 128 + idx]
    ddf = (ddp / DIAG_PASSES).astype(bf)
    ddp = ddp.astype(bf)

    s = np.arange(MC, dtype=np.float64)
    ang = th[:, None] * s[None, :]                     # [N, MC]
    cosv = np.cos(ang)
    sinv = np.sin(ang)
    cb1 = np.zeros((128, 4 * MC), np.float64)
    for c in range(2):
        rows = slice(128 * c, 128 * (c + 1))
        cb1[:, c * MC:(c + 1) * MC] = cosv[rows]
        cb1[:, 2 * MC + c * MC:2 * MC + (c + 1) * MC] = sinv[rows]
    cb1 = cb1.astype(bf)

    rpow = r64[:, None] ** (s[None, :] + 1.0)          # [N, MC]
    rc = rpow * cosv
    rs = rpow * sinv
    cb2 = np.zeros((128, _C2_COLS), np.float64)
    for c in range(2):
        rows = slice(128 * c, 128 * (c + 1))
        cb2[:, _C2_RPOW + c * MC:_C2_RPOW + (c + 1) * MC] = rpow[rows]
        cb2[:, _C2_RC + c * MC:_C2_RC + (c + 1) * MC] = rc[rows]
        cb2[:, _C2_RS + c * MC:_C2_RS + (c + 1) * MC] = rs[rows]
        cb2[:, _C2_RSN + c * MC:_C2_RSN + (c + 1) * MC] = -rs[rows]
    cb2 = cb2.astype(bf)

    rmc = r64 ** MC
    cmc = np.cos(th * MC)
    smc = np.sin(th * MC)
    mm = np.arange(NMC, dtype=np.float64)
    rfm = r64[:, None] ** (mm[None, :] * MC)           # [N, 4]
    angm = th[:, None] * ((mm[None, :] + 1.0) * MC)
    rfc = rfm * np.cos(angm)
    rfs = rfm * np.sin(angm)

    cf_base = np.zeros((128, _CF_COLS), np.float64)
    for c in range(2):
        rows = slice(128 * c, 128 * (c + 1))
        cf_base[:, _CF_RB + c * MC:_CF_RB + (c + 1) * MC] = r64[rows, None]
        cf_base[:, _CF_RMC + c] = rmc[rows]
        cf_base[:, _CF_CMC + c] = cmc[rows]
        cf_base[:, _CF_SMC + c] = smc[rows]
        cf_base[:, _CF_SMCN + c] = -smc[rows]

    in_maps = []
    for core in range(NCORE):
        b, h = core // 2, core % 2
        cf = cf_base.copy()
        if h == 1:   # only second-half cores consume the received carry
            for c in range(2):
                rows = slice(128 * c, 128 * (c + 1))
                cf[:, _CF_RFC + 4 * c:_CF_RFC + 4 * (c + 1)] = rfc[rows]
                cf[:, _CF_RFS + 4 * c:_CF_RFS + 4 * (c + 1)] = rfs[rows]
                cf[:, _CF_RFSN + 4 * c:_CF_RFSN + 4 * (c + 1)] = -rfs[rows]
        if h == 0:   # first-half cores contribute their end state to group b
            cf[:, _CF_GM + b] = 1.0
        cf[:, _CF_PM + b] = 1.0
        cf = cf.astype(np.float32)

        xs = x[b, h * LLOC:(h + 1) * LLOC, :]          # [LLOC, D]
        # device layout: xp[p, m*8*MC + ki*MC + s] = x^T[ki*128+p, m*MC+s]
        xp = np.ascontiguousarray(
            xs.T.reshape(8, 128, NMC, MC).transpose(1, 2, 0, 3).reshape(
                128, NMC * 8 * MC)).astype(bf)
        in_maps.append({
            "xp": xp, "bgp": bgp, "ctp": ctp, "ddp": ddp, "ddf": ddf,
            "cb1": cb1, "cb2": cb2, "cf": cf,
        })
    return in_maps


def kernel(**inputs) -> np.ndarray:
    global LAST_RESULTS
    from concourse.bass_utils import run_bass_kernel_spmd

    if "nc" not in _CACHE:
        _CACHE["nc"] = _build()
    nc = _CACHE["nc"]

    in_maps = _prep(inputs)
    trace = os.environ.get("LRU_TRACE", "0") == "1"
    res = run_bass_kernel_spmd(
        nc, in_maps, core_ids=list(range(NCORE)), trace=trace,
        trace_cores=list(range(NCORE)) if trace else None,
        stitch_traces=trace,
    )
    LAST_RESULTS = res

    out = np.empty((B, L, D), np.float32)
    for core in range(NCORE):
        b, h = core // 2, core % 2
        o = np.asarray(res.results[core]["outT"]).astype(np.float32)
        # o[p, m*8*MC + di*MC + s] = y^T[di*128+p, m*MC+s]
        yT = o.reshape(128, NMC, 8, MC).transpose(2, 0, 1, 3).reshape(D, LLOC)
        out[b, h * LLOC:(h + 1) * LLOC, :] = yT.T
    return out
